# revision 25
# baseline (speedup 1.0000x reference)
"""Trainium2 Bass kernel for nn_CrossAttention_46540265619919.

Cross-attention with gene-axis pre-reduction, causal softmax, residual +
LayerNorm.  Full (unsharded) inputs in, full output out; internally sharded
across 8 NeuronCores as (batch b, L-half h): core c -> b = c//2, h = c%2.
Each core computes 256 output rows [256, 512] independently (softmax reduces
over K and LN reduces over Gt, both fully local to a core).

v5 structure (why it looks the way it does):
- Sync executes each dma_start's semaphore wait inline before programming
  the descriptor, so a dependent DMA on Sync's queue head-of-line-blocks
  every later descriptor. Sync therefore carries only dependency-free reads.
- The pair AllReduce (kred exchange) fences all in-flight DMA for its whole
  duration (~17us quiet, ~43us contended). The kernel quiesces around it:
  pre-CC phase streams only ck/cv/mask/x (done before the trigger), the CC
  runs on an empty fabric, and the entire x_query stream is issued from
  Scalar's queue *behind* the CC-gated kred_out read, so it starts the
  moment the collective completes and then runs uninterrupted at full rate
  (~420-460 GB/s/core observed when fed).
- Reduction trees: Vector runs ~0.93 f32 elem/ns/partition, Pool (gpsimd)
  only ~0.29, so Pool gets one early chunk and small d-slices.

Self-contained: hardcodes all shapes; no sibling imports.
"""

import os
from contextlib import ExitStack

import numpy as np

import concourse.bass as bass
import concourse.tile as tile
from concourse import bacc, mybir
from concourse.bass_utils import run_bass_kernel_spmd

F32 = mybir.dt.float32
F32R = mybir.dt.float32r
BF16 = mybir.dt.bfloat16
AX = mybir.AxisListType
OP = mybir.AluOpType
AF = mybir.ActivationFunctionType

# Problem shape (fixed).
B, L, K, GT, GC, D = 4, 512, 512, 512, 256, 64
NCORES = 8
LLOC = L // 2          # 256 L-rows per core
LT = LLOC // 128       # 2 l-tiles of 128 rows
KC = K // 128          # 4 k-chunks of 128
# reduction chunk sizes per l-tile (sum = GT; power-of-2 for the halving
# tree). Two 64-gene tail chunks: small-pool slots are never reused within a
# tile, so no conditioned-DMA stall, and the tail tree stays short.
XQ_CHUNKS = (128, 128, 128, 64, 64)
# engine per chunk and tile: GpSimd (~0.29 f32 elem/ns vs Vector's 0.93,
# contiguous only — strided d-slices cost another ~30%) gets exactly one
# early-arriving 128-gene chunk per tile; everything else stays on Vector.
XQ_TREE_ENG = (("v", "g", "v", "v", "v"), ("g", "v", "v", "v", "v"))
GC_LOC = GC // 2       # each core of a pair reduces half the key gene axis
MASK_PENALTY = 1.0e9
LN_EPS = 1e-3

LAST_RESULTS = None    # BassKernelResults of the most recent run (for test harness)
_CACHED_NC = None


def _ensure_trace_hook():
    """If NTFF tracing is requested but this image's `antenv` lacks
    `axon_hooks`, synthesize it from trn_boot's ctypes path so
    run_bass_kernel_spmd's trace branch doesn't crash. Best-effort."""
    try:
        import antenv.axon_hooks  # noqa: F401
        return
    except ImportError:
        pass
    try:
        import sys
        import types
        import trn_agent_boot.trn_boot as tb
        import concourse.bass_utils as bu
        hook = tb._ntff_profile_via_ctypes("/opt/axon/libaxon_pjrt.so")
        mod = types.ModuleType("antenv.axon_hooks")
        mod.get_axon_ntff_profile_hook = lambda: hook
        mod.set_axon_ntff_profile_hook = lambda h: None
        sys.modules["antenv.axon_hooks"] = mod
        bu.upload_artifacts = lambda tmpdir: tmpdir  # no fish creds in-container
    except Exception:
        os.environ["BASS_NEVER_TRACE"] = "1"  # fall back: run untraced


def _build_program():
    """Build + compile the per-core SPMD Tile program."""
    nc = bacc.Bacc(
        "TRN2",
        target_bir_lowering=False,
        debug=False,
        num_devices=NCORES,
    )

    xq_d = nc.dram_tensor("xq", [LLOC, GT, D], F32, kind="ExternalInput").ap()
    ck_d = nc.dram_tensor("ck", [K, GC_LOC, D], F32, kind="ExternalInput").ap()
    cv_d = nc.dram_tensor("cv", [K, GT], F32, kind="ExternalInput").ap()
    x_d = nc.dram_tensor("xres", [LLOC, GT], F32, kind="ExternalInput").ap()
    mask_d = nc.dram_tensor("mask", [LLOC, K], F32, kind="ExternalInput").ap()
    out_d = nc.dram_tensor("out", [LLOC, GT], F32, kind="ExternalOutput").ap()

    with tile.TileContext(nc) as tc, ExitStack() as ctx:
        const = ctx.enter_context(tc.tile_pool(name="const", bufs=1))
        # big stream slots (32 KiB/partition) serve ck AND the 128-gene xq
        # chunks; ck trees finish long before the slots are reused, so no
        # conditioned-descriptor ever stalls a ring.
        big = ctx.enter_context(tc.tile_pool(name="big", bufs=4))
        small = ctx.enter_context(tc.tile_pool(name="small", bufs=2))
        work = ctx.enter_context(tc.tile_pool(name="work", bufs=2))
        smalls = ctx.enter_context(tc.tile_pool(name="smalls", bufs=2))
        ps_mm = ctx.enter_context(tc.tile_pool(name="ps_mm", bufs=3, space="PSUM"))
        ps_tp = ctx.enter_context(tc.tile_pool(name="ps_tp", bufs=2, space="PSUM"))
        dram = ctx.enter_context(tc.tile_pool(name="dram", bufs=1, space="DRAM"))

        def reduce_gene_axis(engkey, t, ng, out_ap):
            """Sum t[128, ng, D] over genes into out_ap[128, D] (contiguous
            in-place halving). Vector finishes with one strided reduce;
            GpSimd (no free-axis reduce) halves to one row and copies."""
            if engkey == "v":
                n = ng
                while n > 8:
                    half = n // 2
                    nc.vector.tensor_add(
                        t[:, 0:half, :], t[:, 0:half, :], t[:, half:n, :]
                    )
                    n = half
                nc.vector.tensor_reduce(
                    out_ap, t[:, 0:8, :].rearrange("p g d -> p d g"),
                    axis=AX.X, op=OP.add,
                )
            else:
                n = ng
                while n > 1:
                    half = n // 2
                    nc.gpsimd.tensor_add(
                        t[:, 0:half, :], t[:, 0:half, :], t[:, half:n, :]
                    )
                    n = half
                nc.gpsimd.tensor_copy(out_ap, t[:, 0, :])

        # ---- constants (no deps) ----
        ones = const.tile([128, 128], F32, tag="ones")
        ident = const.tile([128, 128], F32, tag="ident")
        nc.vector.memset(ones[:], 1.0)
        eps_b = const.tile([128, 1], F32, tag="eps_b")
        nc.vector.memset(eps_b[:], LN_EPS)
        nc.gpsimd.affine_select(
            ident[:], ones[:],
            pattern=[[-1, 128]], base=0, channel_multiplier=1,
            compare_op=OP.is_equal, fill=0.0,
        )

        # ---- pre-collective phase: ck stream + kred trees ----
        kred_in = dram.tile([128, KC, D], F32, tag="kred_in")
        kred_out = dram.tile([128, KC, D], F32, tag="kred_out")
        k_reds = []
        for kc in range(KC):
            t = big.tile([128, 128, D], F32, tag="big")
            nc.sync.dma_start(t[:], ck_d[kc * 128:(kc + 1) * 128, :, :])
            k_red = smalls.tile([128, D], F32, tag="k_red", bufs=4)
            reduce_gene_axis("g" if kc == 0 else "v", t, 128, k_red[:])
            k_reds.append(k_red)

        # small dependency-free loads, all on Sync ahead of the kred writes
        cv_sb = const.tile([128, KC, GT], BF16, tag="cv")
        for kc in range(KC):
            cv_stage = smalls.tile([128, GT], F32, tag="cv_stage")
            nc.sync.dma_start(cv_stage[:], cv_d[kc * 128:(kc + 1) * 128, :])
            nc.scalar.copy(cv_sb[:, kc, :], cv_stage[:])
        mask_ts, x_ts = [], []
        for lt in range(LT):
            lsl = slice(lt * 128, (lt + 1) * 128)
            mask_t = smalls.tile([128, K], F32, tag="mask")
            nc.sync.dma_start(mask_t[:], mask_d[lsl, :])
            x_t = smalls.tile([128, GT], F32, tag="x_t")
            nc.sync.dma_start(x_t[:], x_d[lsl, :])
            mask_ts.append(mask_t)
            x_ts.append(x_t)

        # kred writes on Sync AFTER all the dependency-free loads: by the
        # time the ring reaches them their tree semaphores have (nearly)
        # fired, so the head-of-line wait is tiny. (NOT gpsimd: software-DGE
        # transfers execute on the Pool core itself and starved behind its
        # tree work for ~80us in an earlier revision.)
        for kc in range(KC):
            nc.sync.dma_start(kred_in[:, kc, :], k_reds[kc][:])

        nc.gpsimd.collective_compute(
            "AllReduce",
            OP.add,
            replica_groups=[[2 * b, 2 * b + 1] for b in range(B)],
            ins=[kred_in.opt()],
            outs=[kred_out.opt()],
        )
        kred_sb = smalls.tile([128, KC, D], F32, tag="kred_sb")
        nc.scalar.dma_start(kred_sb[:], kred_out[:])

        # xq stream: all on Sync behind the kred writes
        xq_tiles = []           # [lt][gc] -> (tile, ng)
        for lt in range(LT):
            lsl = slice(lt * 128, (lt + 1) * 128)
            tiles = []
            g0 = 0
            for gc, ng in enumerate(XQ_CHUNKS):
                pool, psz, ptag = (
                    (big, 128, "big") if ng == 128 else (small, 64, "small")
                )
                t = pool.tile([128, psz, D], F32, tag=ptag)
                nc.sync.dma_start(t[:, 0:ng, :], xq_d[lsl, g0:g0 + ng, :])
                tiles.append((t, ng))
                g0 += ng
            xq_tiles.append(tiles)

        k_redT = const.tile([64, K], F32, tag="k_redT")
        for kc in range(KC):
            tp = ps_tp.tile([D, 128], F32, tag="tpose")
            nc.tensor.transpose(tp[:], kred_sb[:, kc, :], ident[:])
            nc.scalar.copy(k_redT[:, kc * 128:(kc + 1) * 128], tp[:])

        # ---- per l-tile pipeline ----
        for lt in range(LT):
            lsl = slice(lt * 128, (lt + 1) * 128)
            tail = lt == LT - 1
            epi = nc.vector if tail else nc.gpsimd

            # scores [128, 512] accumulate per gene-chunk in PSUM (fp32:
            # softmax here is argmax-sharp, absolute score error must be tiny)
            ps_s = ps_mm.tile([128, K], F32, tag="mm")
            for gc, (t, ng) in enumerate(xq_tiles[lt]):
                qp = smalls.tile([128, D], F32, tag="qp", bufs=12)
                reduce_gene_axis(XQ_TREE_ENG[lt][gc], t, ng, qp[:])
                tq = ps_tp.tile([D, 128], F32, tag="tpose_q", bufs=3)
                nc.tensor.transpose(tq[:], qp[:], ident[:])
                qT = smalls.tile([D, 128], F32, tag="qT", bufs=12)
                nc.scalar.copy(qT[:], tq[:])
                nc.tensor.matmul(
                    ps_s[:], qT[:], k_redT[:],
                    start=(gc == 0), stop=(gc == len(XQ_CHUNKS) - 1),
                )

            # masked scores: s = scores + mask (mask is 0 / -1e9). GpSimd
            # cannot read PSUM: the non-tail tile stages through SBUF via
            # Scalar so nothing here sits in Vector's queue ahead of tile1
            # trees.
            mask_t = mask_ts[lt]
            s_sb = work.tile([128, K], F32, tag="s_sb")
            if tail:
                nc.vector.scalar_tensor_tensor(
                    s_sb[:], ps_s[:], 1.0, mask_t[:], op0=OP.mult, op1=OP.add
                )
            else:
                s_raw = work.tile([128, K], F32, tag="s_raw")
                nc.scalar.copy(s_raw[:], ps_s[:])
                epi.tensor_add(s_sb[:], s_raw[:], mask_t[:])

            # softmax: negmax, w = exp(s - max) in place over s_sb, denom
            negmax = smalls.tile([128, 1], F32, tag="negmax")
            nc.vector.tensor_reduce(
                negmax[:], s_sb[:], axis=AX.X, op=OP.max, negate=True
            )
            denom = smalls.tile([128, 1], F32, tag="denom")
            nc.scalar.activation(
                s_sb[:], s_sb[:], AF.Exp, bias=negmax[:], scale=1.0,
                accum_out=denom[:],
            )
            recip = smalls.tile([128, 1], F32, tag="recip")
            nc.vector.reciprocal(recip[:], denom[:])

            # w^T chunks [k=128, l=128] via TensorE transpose, stored bf16
            wT = work.tile([128, KC, 128], BF16, tag="wT")
            for kc in range(KC):
                tw = ps_tp.tile([128, 128], F32, tag="tpose")
                nc.tensor.transpose(tw[:], s_sb[:, kc * 128:(kc + 1) * 128], ident[:])
                nc.scalar.copy(wT[:, kc, :], tw[:])

            # attn [128, 512] = w @ cv   (unnormalized, bf16 inputs)
            ps_a = ps_mm.tile([128, GT], F32, tag="mm")
            for kc in range(KC):
                nc.tensor.matmul(
                    ps_a[:], wT[:, kc, :], cv_sb[:, kc, :],
                    start=(kc == 0), stop=(kc == KC - 1),
                )

            # y = attn * recip + x  (AP-scalar ops lower to *Ptr opcodes Pool
            # lacks -> Vector)
            y = work.tile([128, GT], F32, tag="y")
            nc.vector.scalar_tensor_tensor(
                y[:], ps_a[:], recip[:], x_ts[lt][:], op0=OP.mult, op1=OP.add
            )

            # LayerNorm stats -> [mean, var] -> 1/sqrt(var+eps)
            stats = smalls.tile([128, 6], F32, tag="stats")
            nc.vector.bn_stats(stats[:], y[:])
            mv = smalls.tile([128, 2], F32, tag="mv")
            nc.vector.bn_aggr(mv[:], stats[:])
            std = smalls.tile([128, 1], F32, tag="std")
            nc.scalar.activation(std[:], mv[:, 1:2], AF.Sqrt, bias=eps_b[:], scale=1.0)
            rstd = smalls.tile([128, 1], F32, tag="rstd")
            nc.vector.reciprocal(rstd[:], std[:])

            # out = (y - mean) * rstd, in place over y (gamma/beta on host)
            nc.vector.tensor_scalar(
                y[:], y[:], mv[:, 0:1], rstd[:], op0=OP.subtract, op1=OP.mult
            )
            # out write from Scalar: dependent DMAs stay off Sync's queue
            nc.scalar.dma_start(out_d[lsl, :], y[:])

    nc.compile()
    return nc


def _get_nc():
    global _CACHED_NC
    if _CACHED_NC is None:
        _CACHED_NC = _build_program()
    return _CACHED_NC


def _causal_mask(h: int) -> np.ndarray:
    lg = h * LLOC + np.arange(LLOC)[:, None]
    kk = np.arange(K)[None, :]
    return np.where(kk <= lg, 0.0, -MASK_PENALTY).astype(np.float32)


_MASKS = {h: _causal_mask(h) for h in range(2)}


def kernel(x, x_query, context_key, context_value, gamma, beta):
    global LAST_RESULTS
    x = np.asarray(x, np.float32)
    x_query = np.asarray(x_query, np.float32)
    context_key = np.asarray(context_key, np.float32)
    context_value = np.asarray(context_value, np.float32)
    gamma = np.asarray(gamma, np.float32)
    beta = np.asarray(beta, np.float32)

    nc = _get_nc()
    in_maps = []
    for c in range(NCORES):
        b, h = c // 2, c % 2
        sl = slice(h * LLOC, (h + 1) * LLOC)
        in_maps.append({
            "xq": np.ascontiguousarray(x_query[b, sl]),
            "ck": np.ascontiguousarray(context_key[b, :, h * GC_LOC:(h + 1) * GC_LOC]),
            "cv": np.ascontiguousarray(context_value[b]),
            "xres": np.ascontiguousarray(x[b, sl]),
            "mask": _MASKS[h],
        })

    if os.environ.get("KERNEL_TRACE") or os.environ.get("BASS_TRACE"):
        _ensure_trace_hook()
    res = run_bass_kernel_spmd(
        nc,
        in_maps,
        core_ids=list(range(NCORES)),
        trace=bool(os.environ.get("KERNEL_TRACE")),
    )
    LAST_RESULTS = res

    out = np.empty((B, L, GT), np.float32)
    for c, r in enumerate(res.results):
        b, h = c // 2, c % 2
        out[b, h * LLOC:(h + 1) * LLOC] = r["out"]
    # LN affine (gamma/beta broadcast over the last axis) applied on host.
    out = out * gamma + beta
    return out.astype(np.float32)
